# revision 23
# baseline (speedup 1.0000x reference)
"""MoE FFN (64 experts, top-8 w/ null-expert routing, shared expert) on 8 trn2 cores.

Expert-parallel with load-balanced placement: experts are rank-sorted by token
count and dealt round-robin to the 8 cores, so every core gets one expert from
each load octile and the per-position capacity is exact. The host does routing
(fp64 — provably matches the reference's fp32 top-8 selection given the
measured 4.2e-6 minimum decision margin), dispatch-list construction, the token
gather+transpose, and the final combine; it is all O(N*K) index work. The
device does all of the heavy compute: per-expert gated FFN (silu(X Wg) * (X Wu))
Wd over capacity-packed token buffers, with the combine weight applied during
PSUM eviction, plus a data-parallel shard of the shared expert. Matmul operands
are float16 (full PE rate, fp32 PSUM accumulation, ~5e-4 relative error).
"""

import numpy as np

import concourse.bass as bass
import concourse.bacc as bacc
import concourse.tile as tile
from concourse import mybir
from concourse.bass_utils import run_bass_kernel_spmd

# ---- problem constants (hardcoded per contest rules) ----
B, T, D, H = 4, 1024, 512, 1024
N = B * T                     # 4096 tokens
E, K = 64, 8
NUM_NULL = 64
NCORES = 8
EPC = E // NCORES             # 8 experts per core
TSH = N // NCORES             # 512 tokens per core for the shared expert
KC_D = D // 128               # 4 contraction chunks over d_model
KC_H = H // 128               # 8 contraction chunks over d_hidden

F32 = mybir.dt.float32
F16 = mybir.dt.float16
DT_NP = np.float16

# module-level knobs (test.py pokes these; harness uses defaults)
USE_SILU = True
TRACE = False
BENCH_TMPDIR = None
LAST_RESULTS = None

_PROGRAM_CACHE = {}


def build_program(caps):
    """caps[p] = token capacity of expert-position p on every core (experts are
    rank-sorted by load and dealt round-robin, so position p's cap is the max
    count within its rank octile). The program is specialized to the routing."""
    nc = bacc.Bacc("TRN2", target_bir_lowering=False, debug=False,
                   num_devices=NCORES)
    dt = F16
    SLOTS = sum(caps)
    mt = [(c + 127) // 128 for c in caps]
    mt_base = [sum(mt[:p]) for p in range(EPC)]
    base = [sum(caps[:p]) for p in range(EPC)]
    MTS = sum(mt)

    ins = {
        "xg": nc.dram_tensor("xg", [KC_D, 128, SLOTS], dt, kind="ExternalInput"),
        "wgu": nc.dram_tensor("wgu", [EPC, KC_D, 128, 2 * H], dt, kind="ExternalInput"),
        "wd": nc.dram_tensor("wd", [EPC, KC_H, 128, D], dt, kind="ExternalInput"),
        "wslot": nc.dram_tensor("wslot", [128, MTS], F32, kind="ExternalInput"),
        "xs": nc.dram_tensor("xs", [KC_D, 128, TSH], dt, kind="ExternalInput"),
        "wsgu": nc.dram_tensor("wsgu", [KC_D, 128, 2 * H], dt, kind="ExternalInput"),
        "wsd": nc.dram_tensor("wsd", [KC_H, 128, D], dt, kind="ExternalInput"),
    }
    ow = nc.dram_tensor("ow", [SLOTS, D], F32, kind="ExternalOutput")
    ys = nc.dram_tensor("ys", [TSH, D], F32, kind="ExternalOutput")

    with tile.TileContext(nc) as tc:
        with (
            tc.tile_pool(name="wgu_pool", bufs=2) as wgu_pool,
            tc.tile_pool(name="wd_pool", bufs=2) as wd_pool,
            tc.tile_pool(name="xg_pool", bufs=2) as xg_pool,
            tc.tile_pool(name="s_pool", bufs=2) as s_pool,
            tc.tile_pool(name="tmp_pool", bufs=6) as tmp_pool,
            tc.tile_pool(name="o_pool", bufs=6) as o_pool,
            tc.tile_pool(name="const_pool", bufs=1) as const_pool,
            tc.tile_pool(name="l1_psum", bufs=5, space="PSUM") as l1_psum,
            tc.tile_pool(name="l2_psum", bufs=3, space="PSUM") as l2_psum,
        ):
            wslot_sb = const_pool.tile([128, MTS], F32)
            nc.sync.dma_start(out=wslot_sb[:], in_=ins["wslot"][:])


            def ffn(xg_c, wgu_c, wd_sb, ncols, out_dram, scale_cols):
                """gated FFN over ncols tokens: out = (silu(X Wg) * (X Wu)) Wd,
                rows scaled by per-slot combine weights (or None). xg_c/wgu_c are
                per-k-chunk tile lists so the PE can start after the first chunk
                lands."""
                s_sb = s_pool.tile([128, KC_H, ncols], dt, tag="s")
                # fewest 512-max chunks: single chunk when ncols fits one PSUM
                # bank (halves the LDWEIGHTS count for those positions)
                nchunks = (ncols + 511) // 512
                nhalf = (ncols + nchunks - 1) // nchunks
                for j in range(KC_H):
                    for n0 in range(0, ncols, nhalf):
                        nw = min(nhalf, ncols - n0)
                        gp = l1_psum.tile([128, 512], F32, tag="l1")
                        up = l1_psum.tile([128, 512], F32, tag="l1")
                        for c in range(KC_D):
                            nc.tensor.matmul(
                                gp[:, :nw],
                                (wgu_c[c][:, j * 128:(j + 1) * 128]),
                                (xg_c[c][:, n0:n0 + nw]),
                                start=(c == 0), stop=(c == KC_D - 1),
                            )
                        for c in range(KC_D):
                            nc.tensor.matmul(
                                up[:, :nw],
                                (wgu_c[c][:, H + j * 128:H + (j + 1) * 128]),
                                (xg_c[c][:, n0:n0 + nw]),
                                start=(c == 0), stop=(c == KC_D - 1),
                            )
                        tmp = tmp_pool.tile([128, 512], F32, tag="tmp")
                        if USE_SILU:
                            nc.scalar.activation(
                                tmp[:, :nw], gp[:, :nw],
                                mybir.ActivationFunctionType.Silu,
                            )
                            nc.vector.tensor_tensor(
                                out=s_sb[:, j, n0:n0 + nw],
                                in0=tmp[:, :nw], in1=up[:, :nw],
                                op=mybir.AluOpType.mult,
                            )
                        else:
                            # silu(g)*u = (sigmoid(g)*u)*g
                            nc.scalar.activation(
                                tmp[:, :nw], gp[:, :nw],
                                mybir.ActivationFunctionType.Sigmoid,
                            )
                            tmp2 = tmp_pool.tile([128, 512], F32, tag="tmp2")
                            nc.vector.tensor_tensor(
                                out=tmp2[:, :nw],
                                in0=tmp[:, :nw], in1=up[:, :nw],
                                op=mybir.AluOpType.mult,
                            )
                            nc.vector.tensor_tensor(
                                out=s_sb[:, j, n0:n0 + nw],
                                in0=tmp2[:, :nw], in1=gp[:, :nw],
                                op=mybir.AluOpType.mult,
                            )
                for m in range((ncols + 127) // 128):
                    mr = min(128, ncols - m * 128)
                    op_ = l2_psum.tile([128, D], F32, tag="l2")
                    for c in range(KC_H):
                        nc.tensor.matmul(
                            op_[:mr],
                            (s_sb[:, c, m * 128:m * 128 + mr]),
                            (wd_sb[:, c, :]),
                            start=(c == 0), stop=(c == KC_H - 1),
                        )
                    ob = o_pool.tile([128, D], F32, tag="o")
                    if scale_cols is not None:
                        sc = wslot_sb[:, scale_cols + m:scale_cols + m + 1]
                        nc.scalar.activation(
                            ob[:mr], op_[:mr], mybir.ActivationFunctionType.Copy,
                            scale=sc[:mr],
                        )
                    else:
                        nc.scalar.activation(
                            ob[:mr], op_[:mr], mybir.ActivationFunctionType.Copy,
                        )
                    nc.sync.dma_start(
                        out=out_dram[m * 128:m * 128 + mr, :], in_=ob[:mr]
                    )

            def load_chunked(e):
                cap = caps[e] if e is not None else TSH
                xg_c, wgu_c = [], []
                for c in range(KC_D):
                    xt = xg_pool.tile([128, cap], dt, tag=f"xg{c}")
                    if e is not None:
                        nc.sync.dma_start(
                            out=xt[:],
                            in_=ins["xg"][c, :, base[e]:base[e] + cap])
                    else:
                        nc.sync.dma_start(out=xt[:], in_=ins["xs"][c])
                    wt = wgu_pool.tile([128, 2 * H], dt, tag=f"wgu{c}")
                    nc.sync.dma_start(
                        out=wt[:],
                        in_=ins["wgu"][e, c] if e is not None else ins["wsgu"][c])
                    xg_c.append(xt)
                    wgu_c.append(wt)
                wd_sb = wd_pool.tile([128, KC_H, D], dt, tag="wd")
                nc.sync.dma_start(
                    out=wd_sb[:],
                    in_=(ins["wd"][e] if e is not None else ins["wsd"]).rearrange(
                        "c p n -> p c n"))
                return xg_c, wgu_c, wd_sb

            # shared expert runs mid-sequence so the kernel tail ends on a
            # small expert-output DMA instead of the 1MB ys write.
            for e in range(EPC // 2):
                xg_c, wgu_c, wd_sb = load_chunked(e)
                ffn(xg_c, wgu_c, wd_sb, caps[e],
                    ow[base[e]:base[e] + caps[e], :], mt_base[e])
            xs_c, wsgu_c, wsd_sb = load_chunked(None)
            ffn(xs_c, wsgu_c, wsd_sb, TSH, ys, None)
            for e in range(EPC // 2, EPC):
                xg_c, wgu_c, wd_sb = load_chunked(e)
                ffn(xg_c, wgu_c, wd_sb, caps[e],
                    ow[base[e]:base[e] + caps[e], :], mt_base[e])

    nc.compile()
    return nc


# ---------------- host-side routing ----------------

def _route(x, gate_w, logit_bias, null_logit):
    """fp64 router: returns top8 expert ids [N,K], renormalized weights [N,K],
    and the fp64 real logits [N,E] (for the aux loss)."""
    xf = x.reshape(N, D).astype(np.float64)
    gl = xf @ gate_w.astype(np.float64).T + logit_bias.astype(np.float64)  # [N,E]
    nl = float(null_logit)
    # top-8 of [real logits, 64 copies of null_logit]; ties: real (lower idx) wins
    # because top_k prefers the lower index on exact ties.
    top8 = np.argpartition(-gl, K, axis=1)[:, :K]          # candidate top-8 reals
    lv = np.take_along_axis(gl, top8, axis=1)              # their logits
    nreal_above = (gl > nl).sum(axis=1)
    # slots where fewer than K reals beat the null get w=0 (null takes the slot).
    # rank the K candidates per token descending; slot is real iff rank < #"real wins"
    ordr = np.argsort(-lv, axis=1, kind="stable")
    rank = np.empty_like(ordr)
    np.put_along_axis(rank, ordr, np.arange(K)[None, :].repeat(N, 0), axis=1)
    n_take = np.minimum(nreal_above, K)
    sel = rank < n_take[:, None]
    ex = np.exp(lv - lv.max(axis=1, keepdims=True)) * sel
    den = np.clip(ex.sum(axis=1, keepdims=True), 1e-300, None)
    w = ex / den
    return top8, w.astype(np.float64), sel, gl


def _aux_loss(gl, top8, sel, null_logit):
    """faithful fp64 recompute of the reference aux loss."""
    nl = float(null_logit)
    # P_real: mean softmax over the 64 real logits
    exr = np.exp(gl - gl.max(axis=1, keepdims=True))
    P_real = (exr / exr.sum(axis=1, keepdims=True)).mean(axis=0)       # [E]
    # counts: null slots counted as expert 0 then subtracted
    idx = np.where(sel, top8, 0)
    counts = np.bincount(idx.ravel(), minlength=E).astype(np.float64)
    n_null = (~sel).sum()
    counts[0] -= n_null
    f_real = counts / max(counts.sum(), 1e-6)
    L_bal = E * float((f_real * P_real).sum())
    null_rate = n_null / (N * K)
    L_null = (null_rate - 0.5) ** 2
    # lse over [real logits, 64 nulls]
    m = np.maximum(gl.max(axis=1), nl)
    lse = m + np.log(
        np.exp(gl - m[:, None]).sum(axis=1) + NUM_NULL * np.exp(nl - m)
    )
    L_z = float((lse ** 2).mean())
    return np.float32(0.02 * L_bal + 0.001 * L_z + 0.01 * L_null)


def kernel(x, gate_w, logit_bias, null_logit, W_gate, W_up, W_down, Wsg, Wsu, Wsd):
    global LAST_RESULTS
    x = np.asarray(x, np.float32)
    xf = x.reshape(N, D)

    top8, w, sel, gl = _route(x, np.asarray(gate_w), np.asarray(logit_bias),
                              np.asarray(null_logit))
    aux = _aux_loss(gl, top8, sel, np.asarray(null_logit))

    # dispatch: per-expert token lists (order within an expert is irrelevant)
    flat_e = np.where(sel, top8, E).ravel()          # null -> dummy expert E
    flat_w = (w * sel).ravel().astype(np.float32)
    flat_t = np.repeat(np.arange(N, dtype=np.int64), K)
    order = np.argsort(flat_e, kind="stable")
    sorted_e = flat_e[order]
    seg = np.searchsorted(sorted_e, np.arange(E + 1))
    pos_sorted = np.arange(N * K, dtype=np.int64) - seg[np.minimum(sorted_e, E)]
    real = sorted_e < E
    a_idx = order[real]                               # flat (t,k) index
    a_tok = flat_t[a_idx]
    a_w = flat_w[a_idx]
    a_e = sorted_e[real]                              # global expert id
    a_pos = pos_sorted[real]                          # rank within expert

    # load balance: rank-sort experts by count, deal round-robin to cores.
    # expert with global rank r -> core r % 8, position r // 8; the capacity of
    # position p is the largest count in its rank octile (exact, no headroom
    # needed -- counts are known here).
    counts = np.bincount(a_e, minlength=E)
    rank_order = np.argsort(-counts, kind="stable")   # expert ids by desc count
    core_of = np.empty(E, np.int64); pos_of = np.empty(E, np.int64)
    for r, g in enumerate(rank_order):
        core_of[g] = r % NCORES
        pos_of[g] = r // NCORES
    caps = tuple(max(int(counts[rank_order[p * NCORES]]), 128) for p in range(EPC))
    SLOTS = sum(caps)
    mt = [(c + 127) // 128 for c in caps]
    MTS = sum(mt)
    base = np.array([sum(caps[:p]) for p in range(EPC)], np.int64)

    a_slot = (core_of[a_e] * SLOTS + base[pos_of[a_e]] + a_pos)
    slot_idx = np.full((N, K), NCORES * SLOTS, np.int64)   # dummy = zeros row
    slot_idx[a_tok, a_idx % K] = a_slot

    # gather tokens into per-core position-blocked buffers
    tok_by_slot = np.zeros(NCORES * SLOTS, np.int64)
    w_by_slot = np.zeros(NCORES * SLOTS, np.float32)
    used = np.zeros(NCORES * SLOTS, bool)
    tok_by_slot[a_slot] = a_tok
    w_by_slot[a_slot] = a_w
    used[a_slot] = True
    xg_all = xf[tok_by_slot]
    xg_all[~used] = 0.0

    key = ("prog", caps)
    nc = _PROGRAM_CACHE.get(key)
    if nc is None:
        nc = build_program(caps)
        _PROGRAM_CACHE.clear()
        _PROGRAM_CACHE[key] = nc

    wgu_full = np.concatenate(
        [np.asarray(W_gate, np.float32), np.asarray(W_up, np.float32)], axis=2
    ).astype(DT_NP)                                    # [E, D, 2H]
    wgu_full = wgu_full.reshape(E, KC_D, 128, 2 * H)
    wd_full = np.asarray(W_down, np.float32).astype(DT_NP).reshape(E, KC_H, 128, D)
    wsgu = np.concatenate(
        [np.asarray(Wsg, np.float32).T, np.asarray(Wsu, np.float32).T], axis=1
    ).astype(DT_NP).reshape(KC_D, 128, 2 * H).copy()
    wsd = np.ascontiguousarray(np.asarray(Wsd, np.float32).T).astype(DT_NP).reshape(KC_H, 128, D)

    in_maps = []
    for c in range(NCORES):
        sl = slice(c * SLOTS, (c + 1) * SLOTS)
        xg_c = np.ascontiguousarray(xg_all[sl].T).astype(DT_NP).reshape(KC_D, 128, SLOTS)
        ws_pad = np.zeros((MTS, 128), np.float32)
        wc = w_by_slot[sl]
        for p in range(EPC):
            for m in range(mt[p]):
                lo = base[p] + m * 128
                hi = min(lo + 128, base[p] + caps[p])
                ws_pad[sum(mt[:p]) + m, :hi - lo] = wc[lo:hi]
        ws_c = ws_pad.T.copy()                                 # [128, MTS]
        perm = [int(rank_order[p * NCORES + c]) for p in range(EPC)]
        xs_c = np.ascontiguousarray(xf[c * TSH:(c + 1) * TSH].T).astype(
            DT_NP).reshape(KC_D, 128, TSH)
        in_maps.append({
            "xg": xg_c,
            "wgu": np.ascontiguousarray(wgu_full[perm]),
            "wd": np.ascontiguousarray(wd_full[perm]),
            "wslot": ws_c,
            "xs": xs_c,
            "wsgu": wsgu,
            "wsd": wsd,
        })

    res = run_bass_kernel_spmd(
        nc, in_maps, list(range(NCORES)),
        trace=TRACE, tmpdir=BENCH_TMPDIR,
    )
    LAST_RESULTS = res

    # combine: routed[t] = sum over the token's K slots of ow[slot]
    ow_all = np.concatenate(
        [r["ow"] for r in res.results] + [np.zeros((1, D), np.float32)], axis=0
    )
    routed = ow_all[slot_idx[:, 0]]
    for k in range(1, K):
        routed += ow_all[slot_idx[:, k]]
    shared = np.concatenate([r["ys"] for r in res.results], axis=0)  # [N, D]
    y = (shared + routed).reshape(B, T, D)
    return y, aux


# revision 24
# speedup vs baseline: 1.0020x; 1.0020x over previous
"""MoE FFN (64 experts, top-8 w/ null-expert routing, shared expert) on 8 trn2 cores.

Expert-parallel with load-balanced placement: experts are rank-sorted by token
count and dealt round-robin to the 8 cores, so every core gets one expert from
each load octile and the per-position capacity is exact. The host does routing
(fp64 — provably matches the reference's fp32 top-8 selection given the
measured 4.2e-6 minimum decision margin), dispatch-list construction, the token
gather+transpose, and the final combine; it is all O(N*K) index work. The
device does all of the heavy compute: per-expert gated FFN (silu(X Wg) * (X Wu))
Wd over capacity-packed token buffers, with the combine weight applied during
PSUM eviction, plus a data-parallel shard of the shared expert. Matmul operands
are float16 (full PE rate, fp32 PSUM accumulation, ~5e-4 relative error).
"""

import numpy as np

import concourse.bass as bass
import concourse.bacc as bacc
import concourse.tile as tile
from concourse import mybir
from concourse.bass_utils import run_bass_kernel_spmd

# ---- problem constants (hardcoded per contest rules) ----
B, T, D, H = 4, 1024, 512, 1024
N = B * T                     # 4096 tokens
E, K = 64, 8
NUM_NULL = 64
NCORES = 8
EPC = E // NCORES             # 8 experts per core
TSH = N // NCORES             # 512 tokens per core for the shared expert
KC_D = D // 128               # 4 contraction chunks over d_model
KC_H = H // 128               # 8 contraction chunks over d_hidden

F32 = mybir.dt.float32
F16 = mybir.dt.float16
DT_NP = np.float16

# module-level knobs (test.py pokes these; harness uses defaults)
USE_SILU = True
TRACE = False
BENCH_TMPDIR = None
LAST_RESULTS = None

_PROGRAM_CACHE = {}


def build_program(caps):
    """caps[p] = token capacity of expert-position p on every core (experts are
    rank-sorted by load and dealt round-robin, so position p's cap is the max
    count within its rank octile). The program is specialized to the routing."""
    nc = bacc.Bacc("TRN2", target_bir_lowering=False, debug=False,
                   num_devices=NCORES)
    dt = F16
    SLOTS = sum(caps)
    mt = [(c + 127) // 128 for c in caps]
    mt_base = [sum(mt[:p]) for p in range(EPC)]
    base = [sum(caps[:p]) for p in range(EPC)]
    MTS = sum(mt)

    ins = {
        "xg": nc.dram_tensor("xg", [KC_D, 128, SLOTS], dt, kind="ExternalInput"),
        "wgu": nc.dram_tensor("wgu", [EPC, KC_D, 128, 2 * H], dt, kind="ExternalInput"),
        "wd": nc.dram_tensor("wd", [EPC, KC_H, 128, D], dt, kind="ExternalInput"),
        "wslot": nc.dram_tensor("wslot", [128, MTS], F32, kind="ExternalInput"),
        "xs": nc.dram_tensor("xs", [KC_D, 128, TSH], dt, kind="ExternalInput"),
        "wsgu": nc.dram_tensor("wsgu", [KC_D, 128, 2 * H], dt, kind="ExternalInput"),
        "wsd": nc.dram_tensor("wsd", [KC_H, 128, D], dt, kind="ExternalInput"),
    }
    ow = nc.dram_tensor("ow", [SLOTS, D], F32, kind="ExternalOutput")
    ys = nc.dram_tensor("ys", [TSH, D], F32, kind="ExternalOutput")

    with tile.TileContext(nc) as tc:
        with (
            tc.tile_pool(name="wgu_pool", bufs=2) as wgu_pool,
            tc.tile_pool(name="wd_pool", bufs=2) as wd_pool,
            tc.tile_pool(name="xg_pool", bufs=2) as xg_pool,
            tc.tile_pool(name="s_pool", bufs=2) as s_pool,
            tc.tile_pool(name="tmp_pool", bufs=4) as tmp_pool,
            tc.tile_pool(name="o_pool", bufs=4) as o_pool,
            tc.tile_pool(name="const_pool", bufs=1) as const_pool,
            tc.tile_pool(name="l1_psum", bufs=4, space="PSUM") as l1_psum,
            tc.tile_pool(name="l2_psum", bufs=3, space="PSUM") as l2_psum,
        ):
            wslot_sb = const_pool.tile([128, MTS], F32)
            nc.sync.dma_start(out=wslot_sb[:], in_=ins["wslot"][:])


            def ffn(xg_c, wgu_c, wd_sb, ncols, out_dram, scale_cols):
                """gated FFN over ncols tokens: out = (silu(X Wg) * (X Wu)) Wd,
                rows scaled by per-slot combine weights (or None). xg_c/wgu_c are
                per-k-chunk tile lists so the PE can start after the first chunk
                lands."""
                s_sb = s_pool.tile([128, KC_H, ncols], dt, tag="s")
                # fewest 512-max chunks: single chunk when ncols fits one PSUM
                # bank (halves the LDWEIGHTS count for those positions)
                nchunks = (ncols + 511) // 512
                nhalf = (ncols + nchunks - 1) // nchunks
                for j in range(KC_H):
                    for n0 in range(0, ncols, nhalf):
                        nw = min(nhalf, ncols - n0)
                        gp = l1_psum.tile([128, 512], F32, tag="l1")
                        up = l1_psum.tile([128, 512], F32, tag="l1")
                        for c in range(KC_D):
                            nc.tensor.matmul(
                                gp[:, :nw],
                                (wgu_c[c][:, j * 128:(j + 1) * 128]),
                                (xg_c[c][:, n0:n0 + nw]),
                                start=(c == 0), stop=(c == KC_D - 1),
                            )
                        for c in range(KC_D):
                            nc.tensor.matmul(
                                up[:, :nw],
                                (wgu_c[c][:, H + j * 128:H + (j + 1) * 128]),
                                (xg_c[c][:, n0:n0 + nw]),
                                start=(c == 0), stop=(c == KC_D - 1),
                            )
                        tmp = tmp_pool.tile([128, 512], F32, tag="tmp")
                        if USE_SILU:
                            nc.scalar.activation(
                                tmp[:, :nw], gp[:, :nw],
                                mybir.ActivationFunctionType.Silu,
                            )
                            nc.vector.tensor_tensor(
                                out=s_sb[:, j, n0:n0 + nw],
                                in0=tmp[:, :nw], in1=up[:, :nw],
                                op=mybir.AluOpType.mult,
                            )
                        else:
                            # silu(g)*u = (sigmoid(g)*u)*g
                            nc.scalar.activation(
                                tmp[:, :nw], gp[:, :nw],
                                mybir.ActivationFunctionType.Sigmoid,
                            )
                            tmp2 = tmp_pool.tile([128, 512], F32, tag="tmp2")
                            nc.vector.tensor_tensor(
                                out=tmp2[:, :nw],
                                in0=tmp[:, :nw], in1=up[:, :nw],
                                op=mybir.AluOpType.mult,
                            )
                            nc.vector.tensor_tensor(
                                out=s_sb[:, j, n0:n0 + nw],
                                in0=tmp2[:, :nw], in1=gp[:, :nw],
                                op=mybir.AluOpType.mult,
                            )
                for m in range((ncols + 127) // 128):
                    mr = min(128, ncols - m * 128)
                    op_ = l2_psum.tile([128, D], F32, tag="l2")
                    for c in range(KC_H):
                        nc.tensor.matmul(
                            op_[:mr],
                            (s_sb[:, c, m * 128:m * 128 + mr]),
                            (wd_sb[:, c, :]),
                            start=(c == 0), stop=(c == KC_H - 1),
                        )
                    ob = o_pool.tile([128, D], F32, tag="o")
                    if scale_cols is not None:
                        sc = wslot_sb[:, scale_cols + m:scale_cols + m + 1]
                        nc.scalar.activation(
                            ob[:mr], op_[:mr], mybir.ActivationFunctionType.Copy,
                            scale=sc[:mr],
                        )
                    else:
                        nc.scalar.activation(
                            ob[:mr], op_[:mr], mybir.ActivationFunctionType.Copy,
                        )
                    nc.sync.dma_start(
                        out=out_dram[m * 128:m * 128 + mr, :], in_=ob[:mr]
                    )

            def load_chunked(e):
                cap = caps[e] if e is not None else TSH
                xg_c, wgu_c = [], []
                for c in range(KC_D):
                    xt = xg_pool.tile([128, cap], dt, tag=f"xg{c}")
                    if e is not None:
                        nc.sync.dma_start(
                            out=xt[:],
                            in_=ins["xg"][c, :, base[e]:base[e] + cap])
                    else:
                        nc.sync.dma_start(out=xt[:], in_=ins["xs"][c])
                    wt = wgu_pool.tile([128, 2 * H], dt, tag=f"wgu{c}")
                    nc.sync.dma_start(
                        out=wt[:],
                        in_=ins["wgu"][e, c] if e is not None else ins["wsgu"][c])
                    xg_c.append(xt)
                    wgu_c.append(wt)
                wd_sb = wd_pool.tile([128, KC_H, D], dt, tag="wd")
                nc.sync.dma_start(
                    out=wd_sb[:],
                    in_=(ins["wd"][e] if e is not None else ins["wsd"]).rearrange(
                        "c p n -> p c n"))
                return xg_c, wgu_c, wd_sb

            # shared expert runs mid-sequence so the kernel tail ends on a
            # small expert-output DMA instead of the 1MB ys write.
            for e in range(EPC // 2):
                xg_c, wgu_c, wd_sb = load_chunked(e)
                ffn(xg_c, wgu_c, wd_sb, caps[e],
                    ow[base[e]:base[e] + caps[e], :], mt_base[e])
            xs_c, wsgu_c, wsd_sb = load_chunked(None)
            ffn(xs_c, wsgu_c, wsd_sb, TSH, ys, None)
            for e in range(EPC // 2, EPC):
                xg_c, wgu_c, wd_sb = load_chunked(e)
                ffn(xg_c, wgu_c, wd_sb, caps[e],
                    ow[base[e]:base[e] + caps[e], :], mt_base[e])

    nc.compile()
    return nc


# ---------------- host-side routing ----------------

def _route(x, gate_w, logit_bias, null_logit):
    """fp64 router: returns top8 expert ids [N,K], renormalized weights [N,K],
    and the fp64 real logits [N,E] (for the aux loss)."""
    xf = x.reshape(N, D).astype(np.float64)
    gl = xf @ gate_w.astype(np.float64).T + logit_bias.astype(np.float64)  # [N,E]
    nl = float(null_logit)
    # top-8 of [real logits, 64 copies of null_logit]; ties: real (lower idx) wins
    # because top_k prefers the lower index on exact ties.
    top8 = np.argpartition(-gl, K, axis=1)[:, :K]          # candidate top-8 reals
    lv = np.take_along_axis(gl, top8, axis=1)              # their logits
    nreal_above = (gl > nl).sum(axis=1)
    # slots where fewer than K reals beat the null get w=0 (null takes the slot).
    # rank the K candidates per token descending; slot is real iff rank < #"real wins"
    ordr = np.argsort(-lv, axis=1, kind="stable")
    rank = np.empty_like(ordr)
    np.put_along_axis(rank, ordr, np.arange(K)[None, :].repeat(N, 0), axis=1)
    n_take = np.minimum(nreal_above, K)
    sel = rank < n_take[:, None]
    ex = np.exp(lv - lv.max(axis=1, keepdims=True)) * sel
    den = np.clip(ex.sum(axis=1, keepdims=True), 1e-300, None)
    w = ex / den
    return top8, w.astype(np.float64), sel, gl


def _aux_loss(gl, top8, sel, null_logit):
    """faithful fp64 recompute of the reference aux loss."""
    nl = float(null_logit)
    # P_real: mean softmax over the 64 real logits
    exr = np.exp(gl - gl.max(axis=1, keepdims=True))
    P_real = (exr / exr.sum(axis=1, keepdims=True)).mean(axis=0)       # [E]
    # counts: null slots counted as expert 0 then subtracted
    idx = np.where(sel, top8, 0)
    counts = np.bincount(idx.ravel(), minlength=E).astype(np.float64)
    n_null = (~sel).sum()
    counts[0] -= n_null
    f_real = counts / max(counts.sum(), 1e-6)
    L_bal = E * float((f_real * P_real).sum())
    null_rate = n_null / (N * K)
    L_null = (null_rate - 0.5) ** 2
    # lse over [real logits, 64 nulls]
    m = np.maximum(gl.max(axis=1), nl)
    lse = m + np.log(
        np.exp(gl - m[:, None]).sum(axis=1) + NUM_NULL * np.exp(nl - m)
    )
    L_z = float((lse ** 2).mean())
    return np.float32(0.02 * L_bal + 0.001 * L_z + 0.01 * L_null)


def kernel(x, gate_w, logit_bias, null_logit, W_gate, W_up, W_down, Wsg, Wsu, Wsd):
    global LAST_RESULTS
    x = np.asarray(x, np.float32)
    xf = x.reshape(N, D)

    top8, w, sel, gl = _route(x, np.asarray(gate_w), np.asarray(logit_bias),
                              np.asarray(null_logit))
    aux = _aux_loss(gl, top8, sel, np.asarray(null_logit))

    # dispatch: per-expert token lists (order within an expert is irrelevant)
    flat_e = np.where(sel, top8, E).ravel()          # null -> dummy expert E
    flat_w = (w * sel).ravel().astype(np.float32)
    flat_t = np.repeat(np.arange(N, dtype=np.int64), K)
    order = np.argsort(flat_e, kind="stable")
    sorted_e = flat_e[order]
    seg = np.searchsorted(sorted_e, np.arange(E + 1))
    pos_sorted = np.arange(N * K, dtype=np.int64) - seg[np.minimum(sorted_e, E)]
    real = sorted_e < E
    a_idx = order[real]                               # flat (t,k) index
    a_tok = flat_t[a_idx]
    a_w = flat_w[a_idx]
    a_e = sorted_e[real]                              # global expert id
    a_pos = pos_sorted[real]                          # rank within expert

    # load balance: rank-sort experts by count, deal round-robin to cores.
    # expert with global rank r -> core r % 8, position r // 8; the capacity of
    # position p is the largest count in its rank octile (exact, no headroom
    # needed -- counts are known here).
    counts = np.bincount(a_e, minlength=E)
    rank_order = np.argsort(-counts, kind="stable")   # expert ids by desc count
    core_of = np.empty(E, np.int64); pos_of = np.empty(E, np.int64)
    for r, g in enumerate(rank_order):
        core_of[g] = r % NCORES
        pos_of[g] = r // NCORES
    caps = tuple(max(int(counts[rank_order[p * NCORES]]), 128) for p in range(EPC))
    SLOTS = sum(caps)
    mt = [(c + 127) // 128 for c in caps]
    MTS = sum(mt)
    base = np.array([sum(caps[:p]) for p in range(EPC)], np.int64)

    a_slot = (core_of[a_e] * SLOTS + base[pos_of[a_e]] + a_pos)
    slot_idx = np.full((N, K), NCORES * SLOTS, np.int64)   # dummy = zeros row
    slot_idx[a_tok, a_idx % K] = a_slot

    # gather tokens into per-core position-blocked buffers
    tok_by_slot = np.zeros(NCORES * SLOTS, np.int64)
    w_by_slot = np.zeros(NCORES * SLOTS, np.float32)
    used = np.zeros(NCORES * SLOTS, bool)
    tok_by_slot[a_slot] = a_tok
    w_by_slot[a_slot] = a_w
    used[a_slot] = True
    xg_all = xf[tok_by_slot]
    xg_all[~used] = 0.0

    key = ("prog", caps)
    nc = _PROGRAM_CACHE.get(key)
    if nc is None:
        nc = build_program(caps)
        _PROGRAM_CACHE.clear()
        _PROGRAM_CACHE[key] = nc

    wgu_full = np.concatenate(
        [np.asarray(W_gate, np.float32), np.asarray(W_up, np.float32)], axis=2
    ).astype(DT_NP)                                    # [E, D, 2H]
    wgu_full = wgu_full.reshape(E, KC_D, 128, 2 * H)
    wd_full = np.asarray(W_down, np.float32).astype(DT_NP).reshape(E, KC_H, 128, D)
    wsgu = np.concatenate(
        [np.asarray(Wsg, np.float32).T, np.asarray(Wsu, np.float32).T], axis=1
    ).astype(DT_NP).reshape(KC_D, 128, 2 * H).copy()
    wsd = np.ascontiguousarray(np.asarray(Wsd, np.float32).T).astype(DT_NP).reshape(KC_H, 128, D)

    in_maps = []
    for c in range(NCORES):
        sl = slice(c * SLOTS, (c + 1) * SLOTS)
        xg_c = np.ascontiguousarray(xg_all[sl].T).astype(DT_NP).reshape(KC_D, 128, SLOTS)
        ws_pad = np.zeros((MTS, 128), np.float32)
        wc = w_by_slot[sl]
        for p in range(EPC):
            for m in range(mt[p]):
                lo = base[p] + m * 128
                hi = min(lo + 128, base[p] + caps[p])
                ws_pad[sum(mt[:p]) + m, :hi - lo] = wc[lo:hi]
        ws_c = ws_pad.T.copy()                                 # [128, MTS]
        perm = [int(rank_order[p * NCORES + c]) for p in range(EPC)]
        xs_c = np.ascontiguousarray(xf[c * TSH:(c + 1) * TSH].T).astype(
            DT_NP).reshape(KC_D, 128, TSH)
        in_maps.append({
            "xg": xg_c,
            "wgu": np.ascontiguousarray(wgu_full[perm]),
            "wd": np.ascontiguousarray(wd_full[perm]),
            "wslot": ws_c,
            "xs": xs_c,
            "wsgu": wsgu,
            "wsd": wsd,
        })

    res = run_bass_kernel_spmd(
        nc, in_maps, list(range(NCORES)),
        trace=TRACE, tmpdir=BENCH_TMPDIR,
    )
    LAST_RESULTS = res

    # combine: routed[t] = sum over the token's K slots of ow[slot]
    ow_all = np.concatenate(
        [r["ow"] for r in res.results] + [np.zeros((1, D), np.float32)], axis=0
    )
    routed = ow_all[slot_idx[:, 0]]
    for k in range(1, K):
        routed += ow_all[slot_idx[:, k]]
    shared = np.concatenate([r["ys"] for r in res.results], axis=0)  # [N, D]
    y = (shared + routed).reshape(B, T, D)
    return y, aux


# revision 25
# speedup vs baseline: 1.0033x; 1.0013x over previous
"""MoE FFN (64 experts, top-8 w/ null-expert routing, shared expert) on 8 trn2 cores.

Expert-parallel with load-balanced placement: experts are rank-sorted by token
count and dealt round-robin to the 8 cores, so every core gets one expert from
each load octile and the per-position capacity is exact. The host does routing
(fp64 — provably matches the reference's fp32 top-8 selection given the
measured 4.2e-6 minimum decision margin), dispatch-list construction, the token
gather+transpose, and the final combine; it is all O(N*K) index work. The
device does all of the heavy compute: per-expert gated FFN (silu(X Wg) * (X Wu))
Wd over capacity-packed token buffers, with the combine weight applied during
PSUM eviction, plus a data-parallel shard of the shared expert. Matmul operands
are float16 (full PE rate, fp32 PSUM accumulation, ~5e-4 relative error).
"""

import numpy as np

import concourse.bass as bass
import concourse.bacc as bacc
import concourse.tile as tile
from concourse import mybir
from concourse.bass_utils import run_bass_kernel_spmd

# ---- problem constants (hardcoded per contest rules) ----
B, T, D, H = 4, 1024, 512, 1024
N = B * T                     # 4096 tokens
E, K = 64, 8
NUM_NULL = 64
NCORES = 8
EPC = E // NCORES             # 8 experts per core
TSH = N // NCORES             # 512 tokens per core for the shared expert
KC_D = D // 128               # 4 contraction chunks over d_model
KC_H = H // 128               # 8 contraction chunks over d_hidden

F32 = mybir.dt.float32
F16 = mybir.dt.float16
DT_NP = np.float16

# module-level knobs (test.py pokes these; harness uses defaults)
USE_SILU = True
TRACE = False
BENCH_TMPDIR = None
LAST_RESULTS = None

_PROGRAM_CACHE = {}


def build_program(caps):
    """caps[p] = token capacity of expert-position p on every core (experts are
    rank-sorted by load and dealt round-robin, so position p's cap is the max
    count within its rank octile). The program is specialized to the routing."""
    nc = bacc.Bacc("TRN2", target_bir_lowering=False, debug=False,
                   num_devices=NCORES)
    dt = F16
    SLOTS = sum(caps)
    mt = [(c + 127) // 128 for c in caps]
    mt_base = [sum(mt[:p]) for p in range(EPC)]
    base = [sum(caps[:p]) for p in range(EPC)]
    MTS = sum(mt)

    ins = {
        "xg": nc.dram_tensor("xg", [KC_D, 128, SLOTS], dt, kind="ExternalInput"),
        "wgu": nc.dram_tensor("wgu", [EPC, KC_D, 128, 2 * H], dt, kind="ExternalInput"),
        "wd": nc.dram_tensor("wd", [EPC, KC_H, 128, D], dt, kind="ExternalInput"),
        "wslot": nc.dram_tensor("wslot", [128, MTS], F32, kind="ExternalInput"),
        "xs": nc.dram_tensor("xs", [KC_D, 128, TSH], dt, kind="ExternalInput"),
        "wsgu": nc.dram_tensor("wsgu", [KC_D, 128, 2 * H], dt, kind="ExternalInput"),
        "wsd": nc.dram_tensor("wsd", [KC_H, 128, D], dt, kind="ExternalInput"),
    }
    ow = nc.dram_tensor("ow", [SLOTS, D], F32, kind="ExternalOutput")
    ys = nc.dram_tensor("ys", [TSH, D], F32, kind="ExternalOutput")

    with tile.TileContext(nc) as tc:
        with (
            tc.tile_pool(name="wgu_pool", bufs=2) as wgu_pool,
            tc.tile_pool(name="wd_pool", bufs=2) as wd_pool,
            tc.tile_pool(name="xg_pool", bufs=2) as xg_pool,
            tc.tile_pool(name="s_pool", bufs=2) as s_pool,
            tc.tile_pool(name="tmp_pool", bufs=4) as tmp_pool,
            tc.tile_pool(name="o_pool", bufs=4) as o_pool,
            tc.tile_pool(name="const_pool", bufs=1) as const_pool,
            tc.tile_pool(name="l1_psum", bufs=4, space="PSUM") as l1_psum,
            tc.tile_pool(name="l2_psum", bufs=3, space="PSUM") as l2_psum,
        ):
            wslot_sb = const_pool.tile([128, MTS], F32)
            nc.sync.dma_start(out=wslot_sb[:], in_=ins["wslot"][:])


            def ffn(xg_c, wgu_c, wd_sb, ncols, out_dram, scale_cols):
                wg_c, wu_c = wgu_c
                """gated FFN over ncols tokens: out = (silu(X Wg) * (X Wu)) Wd,
                rows scaled by per-slot combine weights (or None). xg_c/wgu_c are
                per-k-chunk tile lists so the PE can start after the first chunk
                lands."""
                s_sb = s_pool.tile([128, KC_H, ncols], dt, tag="s")
                # fewest 512-max chunks: single chunk when ncols fits one PSUM
                # bank (halves the LDWEIGHTS count for those positions)
                nchunks = (ncols + 511) // 512
                nhalf = (ncols + nchunks - 1) // nchunks
                for j in range(KC_H):
                    for n0 in range(0, ncols, nhalf):
                        nw = min(nhalf, ncols - n0)
                        gp = l1_psum.tile([128, 512], F32, tag="l1")
                        up = l1_psum.tile([128, 512], F32, tag="l1")
                        for c in range(KC_D):
                            nc.tensor.matmul(
                                gp[:, :nw],
                                (wg_c[c][:, j * 128:(j + 1) * 128]),
                                (xg_c[c][:, n0:n0 + nw]),
                                start=(c == 0), stop=(c == KC_D - 1),
                            )
                        for c in range(KC_D):
                            nc.tensor.matmul(
                                up[:, :nw],
                                (wu_c[c][:, j * 128:(j + 1) * 128]),
                                (xg_c[c][:, n0:n0 + nw]),
                                start=(c == 0), stop=(c == KC_D - 1),
                            )
                        tmp = tmp_pool.tile([128, 512], F32, tag="tmp")
                        if USE_SILU:
                            nc.scalar.activation(
                                tmp[:, :nw], gp[:, :nw],
                                mybir.ActivationFunctionType.Silu,
                            )
                            nc.vector.tensor_tensor(
                                out=s_sb[:, j, n0:n0 + nw],
                                in0=tmp[:, :nw], in1=up[:, :nw],
                                op=mybir.AluOpType.mult,
                            )
                        else:
                            # silu(g)*u = (sigmoid(g)*u)*g
                            nc.scalar.activation(
                                tmp[:, :nw], gp[:, :nw],
                                mybir.ActivationFunctionType.Sigmoid,
                            )
                            tmp2 = tmp_pool.tile([128, 512], F32, tag="tmp2")
                            nc.vector.tensor_tensor(
                                out=tmp2[:, :nw],
                                in0=tmp[:, :nw], in1=up[:, :nw],
                                op=mybir.AluOpType.mult,
                            )
                            nc.vector.tensor_tensor(
                                out=s_sb[:, j, n0:n0 + nw],
                                in0=tmp2[:, :nw], in1=gp[:, :nw],
                                op=mybir.AluOpType.mult,
                            )
                for m in range((ncols + 127) // 128):
                    mr = min(128, ncols - m * 128)
                    op_ = l2_psum.tile([128, D], F32, tag="l2")
                    for c in range(KC_H):
                        nc.tensor.matmul(
                            op_[:mr],
                            (s_sb[:, c, m * 128:m * 128 + mr]),
                            (wd_sb[:, c, :]),
                            start=(c == 0), stop=(c == KC_H - 1),
                        )
                    ob = o_pool.tile([128, D], F32, tag="o")
                    if scale_cols is not None:
                        sc = wslot_sb[:, scale_cols + m:scale_cols + m + 1]
                        nc.scalar.activation(
                            ob[:mr], op_[:mr], mybir.ActivationFunctionType.Copy,
                            scale=sc[:mr],
                        )
                    else:
                        nc.scalar.activation(
                            ob[:mr], op_[:mr], mybir.ActivationFunctionType.Copy,
                        )
                    nc.sync.dma_start(
                        out=out_dram[m * 128:m * 128 + mr, :], in_=ob[:mr]
                    )

            def load_chunked(e):
                cap = caps[e] if e is not None else TSH
                xg_c, wg_c, wu_c = [], [], []
                for c in range(KC_D):
                    xt = xg_pool.tile([128, cap], dt, tag=f"xg{c}")
                    if e is not None:
                        nc.sync.dma_start(
                            out=xt[:],
                            in_=ins["xg"][c, :, base[e]:base[e] + cap])
                    else:
                        nc.sync.dma_start(out=xt[:], in_=ins["xs"][c])
                    src_wgu = ins["wgu"][e, c] if e is not None else ins["wsgu"][c]
                    gt = wgu_pool.tile([128, H], dt, tag=f"wg{c}")
                    nc.sync.dma_start(out=gt[:], in_=src_wgu[:, :H])
                    xg_c.append(xt)
                    wg_c.append(gt)
                for c in range(KC_D):
                    src_wgu = ins["wgu"][e, c] if e is not None else ins["wsgu"][c]
                    ut = wgu_pool.tile([128, H], dt, tag=f"wu{c}")
                    nc.sync.dma_start(out=ut[:], in_=src_wgu[:, H:])
                    wu_c.append(ut)
                wd_sb = wd_pool.tile([128, KC_H, D], dt, tag="wd")
                nc.sync.dma_start(
                    out=wd_sb[:],
                    in_=(ins["wd"][e] if e is not None else ins["wsd"]).rearrange(
                        "c p n -> p c n"))
                return xg_c, (wg_c, wu_c), wd_sb

            # shared expert runs mid-sequence so the kernel tail ends on a
            # small expert-output DMA instead of the 1MB ys write.
            for e in range(EPC // 2):
                xg_c, wgu_c, wd_sb = load_chunked(e)
                ffn(xg_c, wgu_c, wd_sb, caps[e],
                    ow[base[e]:base[e] + caps[e], :], mt_base[e])
            xs_c, wsgu_c, wsd_sb = load_chunked(None)
            ffn(xs_c, wsgu_c, wsd_sb, TSH, ys, None)
            for e in range(EPC // 2, EPC):
                xg_c, wgu_c, wd_sb = load_chunked(e)
                ffn(xg_c, wgu_c, wd_sb, caps[e],
                    ow[base[e]:base[e] + caps[e], :], mt_base[e])

    nc.compile()
    return nc


# ---------------- host-side routing ----------------

def _route(x, gate_w, logit_bias, null_logit):
    """fp64 router: returns top8 expert ids [N,K], renormalized weights [N,K],
    and the fp64 real logits [N,E] (for the aux loss)."""
    xf = x.reshape(N, D).astype(np.float64)
    gl = xf @ gate_w.astype(np.float64).T + logit_bias.astype(np.float64)  # [N,E]
    nl = float(null_logit)
    # top-8 of [real logits, 64 copies of null_logit]; ties: real (lower idx) wins
    # because top_k prefers the lower index on exact ties.
    top8 = np.argpartition(-gl, K, axis=1)[:, :K]          # candidate top-8 reals
    lv = np.take_along_axis(gl, top8, axis=1)              # their logits
    nreal_above = (gl > nl).sum(axis=1)
    # slots where fewer than K reals beat the null get w=0 (null takes the slot).
    # rank the K candidates per token descending; slot is real iff rank < #"real wins"
    ordr = np.argsort(-lv, axis=1, kind="stable")
    rank = np.empty_like(ordr)
    np.put_along_axis(rank, ordr, np.arange(K)[None, :].repeat(N, 0), axis=1)
    n_take = np.minimum(nreal_above, K)
    sel = rank < n_take[:, None]
    ex = np.exp(lv - lv.max(axis=1, keepdims=True)) * sel
    den = np.clip(ex.sum(axis=1, keepdims=True), 1e-300, None)
    w = ex / den
    return top8, w.astype(np.float64), sel, gl


def _aux_loss(gl, top8, sel, null_logit):
    """faithful fp64 recompute of the reference aux loss."""
    nl = float(null_logit)
    # P_real: mean softmax over the 64 real logits
    exr = np.exp(gl - gl.max(axis=1, keepdims=True))
    P_real = (exr / exr.sum(axis=1, keepdims=True)).mean(axis=0)       # [E]
    # counts: null slots counted as expert 0 then subtracted
    idx = np.where(sel, top8, 0)
    counts = np.bincount(idx.ravel(), minlength=E).astype(np.float64)
    n_null = (~sel).sum()
    counts[0] -= n_null
    f_real = counts / max(counts.sum(), 1e-6)
    L_bal = E * float((f_real * P_real).sum())
    null_rate = n_null / (N * K)
    L_null = (null_rate - 0.5) ** 2
    # lse over [real logits, 64 nulls]
    m = np.maximum(gl.max(axis=1), nl)
    lse = m + np.log(
        np.exp(gl - m[:, None]).sum(axis=1) + NUM_NULL * np.exp(nl - m)
    )
    L_z = float((lse ** 2).mean())
    return np.float32(0.02 * L_bal + 0.001 * L_z + 0.01 * L_null)


def kernel(x, gate_w, logit_bias, null_logit, W_gate, W_up, W_down, Wsg, Wsu, Wsd):
    global LAST_RESULTS
    x = np.asarray(x, np.float32)
    xf = x.reshape(N, D)

    top8, w, sel, gl = _route(x, np.asarray(gate_w), np.asarray(logit_bias),
                              np.asarray(null_logit))
    aux = _aux_loss(gl, top8, sel, np.asarray(null_logit))

    # dispatch: per-expert token lists (order within an expert is irrelevant)
    flat_e = np.where(sel, top8, E).ravel()          # null -> dummy expert E
    flat_w = (w * sel).ravel().astype(np.float32)
    flat_t = np.repeat(np.arange(N, dtype=np.int64), K)
    order = np.argsort(flat_e, kind="stable")
    sorted_e = flat_e[order]
    seg = np.searchsorted(sorted_e, np.arange(E + 1))
    pos_sorted = np.arange(N * K, dtype=np.int64) - seg[np.minimum(sorted_e, E)]
    real = sorted_e < E
    a_idx = order[real]                               # flat (t,k) index
    a_tok = flat_t[a_idx]
    a_w = flat_w[a_idx]
    a_e = sorted_e[real]                              # global expert id
    a_pos = pos_sorted[real]                          # rank within expert

    # load balance: rank-sort experts by count, deal round-robin to cores.
    # expert with global rank r -> core r % 8, position r // 8; the capacity of
    # position p is the largest count in its rank octile (exact, no headroom
    # needed -- counts are known here).
    counts = np.bincount(a_e, minlength=E)
    rank_order = np.argsort(-counts, kind="stable")   # expert ids by desc count
    core_of = np.empty(E, np.int64); pos_of = np.empty(E, np.int64)
    for r, g in enumerate(rank_order):
        core_of[g] = r % NCORES
        pos_of[g] = r // NCORES
    caps = tuple(max(int(counts[rank_order[p * NCORES]]), 128) for p in range(EPC))
    SLOTS = sum(caps)
    mt = [(c + 127) // 128 for c in caps]
    MTS = sum(mt)
    base = np.array([sum(caps[:p]) for p in range(EPC)], np.int64)

    a_slot = (core_of[a_e] * SLOTS + base[pos_of[a_e]] + a_pos)
    slot_idx = np.full((N, K), NCORES * SLOTS, np.int64)   # dummy = zeros row
    slot_idx[a_tok, a_idx % K] = a_slot

    # gather tokens into per-core position-blocked buffers
    tok_by_slot = np.zeros(NCORES * SLOTS, np.int64)
    w_by_slot = np.zeros(NCORES * SLOTS, np.float32)
    used = np.zeros(NCORES * SLOTS, bool)
    tok_by_slot[a_slot] = a_tok
    w_by_slot[a_slot] = a_w
    used[a_slot] = True
    xg_all = xf[tok_by_slot]
    xg_all[~used] = 0.0

    key = ("prog", caps)
    nc = _PROGRAM_CACHE.get(key)
    if nc is None:
        nc = build_program(caps)
        _PROGRAM_CACHE.clear()
        _PROGRAM_CACHE[key] = nc

    wgu_full = np.concatenate(
        [np.asarray(W_gate, np.float32), np.asarray(W_up, np.float32)], axis=2
    ).astype(DT_NP)                                    # [E, D, 2H]
    wgu_full = wgu_full.reshape(E, KC_D, 128, 2 * H)
    wd_full = np.asarray(W_down, np.float32).astype(DT_NP).reshape(E, KC_H, 128, D)
    wsgu = np.concatenate(
        [np.asarray(Wsg, np.float32).T, np.asarray(Wsu, np.float32).T], axis=1
    ).astype(DT_NP).reshape(KC_D, 128, 2 * H).copy()
    wsd = np.ascontiguousarray(np.asarray(Wsd, np.float32).T).astype(DT_NP).reshape(KC_H, 128, D)

    in_maps = []
    for c in range(NCORES):
        sl = slice(c * SLOTS, (c + 1) * SLOTS)
        xg_c = np.ascontiguousarray(xg_all[sl].T).astype(DT_NP).reshape(KC_D, 128, SLOTS)
        ws_pad = np.zeros((MTS, 128), np.float32)
        wc = w_by_slot[sl]
        for p in range(EPC):
            for m in range(mt[p]):
                lo = base[p] + m * 128
                hi = min(lo + 128, base[p] + caps[p])
                ws_pad[sum(mt[:p]) + m, :hi - lo] = wc[lo:hi]
        ws_c = ws_pad.T.copy()                                 # [128, MTS]
        perm = [int(rank_order[p * NCORES + c]) for p in range(EPC)]
        xs_c = np.ascontiguousarray(xf[c * TSH:(c + 1) * TSH].T).astype(
            DT_NP).reshape(KC_D, 128, TSH)
        in_maps.append({
            "xg": xg_c,
            "wgu": np.ascontiguousarray(wgu_full[perm]),
            "wd": np.ascontiguousarray(wd_full[perm]),
            "wslot": ws_c,
            "xs": xs_c,
            "wsgu": wsgu,
            "wsd": wsd,
        })

    res = run_bass_kernel_spmd(
        nc, in_maps, list(range(NCORES)),
        trace=TRACE, tmpdir=BENCH_TMPDIR,
    )
    LAST_RESULTS = res

    # combine: routed[t] = sum over the token's K slots of ow[slot]
    ow_all = np.concatenate(
        [r["ow"] for r in res.results] + [np.zeros((1, D), np.float32)], axis=0
    )
    routed = ow_all[slot_idx[:, 0]]
    for k in range(1, K):
        routed += ow_all[slot_idx[:, k]]
    shared = np.concatenate([r["ys"] for r in res.results], axis=0)  # [N, D]
    y = (shared + routed).reshape(B, T, D)
    return y, aux
